# revision 1
# baseline (speedup 1.0000x reference)
"""LowPassMSELoss Trainium2 kernel.

Math: loss = mean((lfilter(b,a,o) - lfilter(b,a,t))^2)
    = mean(lfilter(b,a,o-t)^2)               [filter is linear]
    = mean(conv(o-t, h)^2)                   [h = impulse response, truncated
                                              to K=128 taps; max pole radius
                                              0.869 -> tail < 2e-8]

Layout per core (2 rows of T=262144):
  - natural load [128, 2048] per row (partition p = contiguous 2048-sample chunk)
  - d = o - t on DVE
  - PE-transpose 128x128 tiles of d -> Xb[i, c] = d[2048*p + 128*tt + i] at
    col c = 128*tt + p ... actually col index c = 512*g + 128*q + p for the
    4-transpose groups; within a chunk p, consecutive 128-blocks are 128 cols
    apart, so block at col c has its time-predecessor at col c-128; the
    first 128 cols get a copied "previous block" pad (exact, not approximate).
  - conv: psum[j, n] = sum_i A[i,j] Xb[i,n] + sum_i B[i,j] XbPrev[i,n]
    (two accumulating matmuls, Toeplitz lhsT built host-side from h)
  - square+reduce per psum tile -> per-partition partial sums [128, 1]
  - host: sum partials over 8 cores / (16*262144)
"""

import os
import numpy as np

B, T = 16, 262144
NCORES = 8
ROWS_PER_CORE = B // NCORES          # 2
F = 2048                             # free dim of natural layout (T / 128)
NT = F // 128                        # 16 transpose tiles per row
K = 128                              # FIR taps
NJ = F // 512                        # 4 conv output tiles per row
XBW = 128 + F                        # XbFull width (128 pad cols + data)

last_exec_time_ns = None
_CACHE = {}


def _impulse_response(b, a, n):
    """First n samples of the IIR impulse response, float64, DF2T like scipy."""
    b = np.asarray(b, np.float64)
    a = np.asarray(a, np.float64)
    b = b / a[0]
    a = a / a[0]
    order = len(a) - 1
    z = np.zeros(order, np.float64)
    h = np.empty(n, np.float64)
    for i in range(n):
        x = 1.0 if i == 0 else 0.0
        y = b[0] * x + z[0]
        znew = np.empty(order, np.float64)
        znew[: order - 1] = z[1:] + b[1:order] * x - a[1:order] * y
        znew[order - 1] = b[order] * x - a[order] * y
        z = znew
        h[i] = y
    return h


def _toeplitz_lhsts(h):
    """lhsT_A[i,j] = h[j-i] (j>=i), lhsT_B[i,j] = h[128+j-i] (i>j).

    y[128n+j] = sum_{i<=j} h[j-i]*cur[i] + sum_{i>j} h[128+j-i]*prev[i]
    matmul(out, lhsT, rhs): out[j, n] = sum_i lhsT[i, j] * rhs[i, n]
    """
    i = np.arange(K)[:, None]
    j = np.arange(K)[None, :]
    dj = j - i
    A = np.where(dj >= 0, h[np.clip(dj, 0, K - 1)], 0.0)
    Bm = np.where(dj < 0, h[np.clip(K + dj, 0, K - 1)], 0.0)
    return A.astype(np.float32), Bm.astype(np.float32)


def _drop_vacuous_self_waits(nc):
    """trn2 codegen allows one sync-wait per instruction; Tile sometimes
    attaches a same-engine self-wait alongside a foreign one.  Engine queues
    issue in order and every same-engine op increments the engine sem, so a
    self-wait whose threshold is already guaranteed by queue position is
    droppable.  (All our PE ops are matmuls — completion is pc-monotone —
    and ACT/DVE execute ops serially, so position implies completion for
    same-engine PSUM/SBUF hazards here.)"""
    import copy

    prior_incs = {}
    for f in nc.m.functions:
        for bb in f.blocks:
            new_list = []
            for ins in bb.instructions:
                si = ins.sync_info
                if (
                    si is not None
                    and si.on_wait
                    and len(si.on_wait) > 1
                    and "Drain" in type(ins).__name__
                ):
                    waits = list(si.on_wait)
                    for k, w in enumerate(waits[:-1]):
                        pre = copy.deepcopy(ins)
                        pre.name = f"{ins.name}-w{k}"
                        pre.sync_info = copy.deepcopy(si)
                        pre.sync_info.on_wait = [w]
                        pre.sync_info.on_update = []
                        new_list.append(pre)
                    si.on_wait = [waits[-1]]
                new_list.append(ins)
            bb.instructions = new_list
    for f in nc.m.functions:
        for bb in f.blocks:
            for ins in bb.instructions:
                si = ins.sync_info
                if si is None:
                    continue
                waits = list(si.on_wait or [])
                if len(waits) > 1:
                    kept = []
                    for w in waits:
                        name = getattr(w, "ant_name", "") or ""
                        eng = getattr(getattr(ins, "engine", None), "value", "zz")
                        if (
                            name.startswith(eng)
                            and prior_incs.get(name, 0) >= (w.wait_value or 0)
                        ):
                            continue
                        kept.append(w)
                    si.on_wait = kept
                for u in si.on_update or []:
                    name = getattr(u, "ant_name", "") or ""
                    if name:
                        prior_incs[name] = prior_incs.get(name, 0) + (
                            u.update_value or 1
                        )


def _build_bass():
    import concourse.bass as bass
    import concourse.tile as tile
    from concourse import mybir

    dt = mybir.dt
    nc = bass.Bass(trn_type="TRN2")

    ot_h = nc.dram_tensor(
        "ot", [ROWS_PER_CORE, 2, T], dt.float32, kind="ExternalInput"
    )
    C_h = nc.dram_tensor("consts", [3, K, K], dt.float32, kind="ExternalInput")
    out_h = nc.dram_tensor(
        "partials", [128, ROWS_PER_CORE * NJ], dt.float32, kind="ExternalOutput"
    )

    # ot4[r, p, s, f] = ot[r, s, 2048p + f]: per row ONE dma -> one sem lane
    ot4 = ot_h[:].rearrange("b s (p f) -> b p s f", p=128)

    with tile.TileContext(nc) as tc:
        with (
            tc.tile_pool(name="consts", bufs=1) as consts,
            tc.tile_pool(name="io", bufs=2) as io_pool,
            tc.tile_pool(name="dpool", bufs=2) as dpool,
            tc.tile_pool(name="xb", bufs=2) as xbpool,
            tc.tile_pool(name="ptr", bufs=2, space="PSUM") as ptr_pool,
            tc.tile_pool(name="pconv", bufs=4, space="PSUM") as pconv_pool,
            tc.tile_pool(name="scr", bufs=2) as scr_pool,
            tc.tile_pool(name="outp", bufs=1) as out_pool,
        ):
            c_raw = consts.tile([K, 3, K], dt.float32, tag="Craw")
            nc.sync.dma_start(c_raw[:], C_h[:].rearrange("c p f -> p c f"))
            # funnel the const-DMA dep through DVE so PE ops wait on one engine
            c_sb = consts.tile([K, 3, K], dt.float32, tag="C")
            nc.vector.tensor_copy(c_sb[:], c_raw[:])
            A_sb = c_sb[:, 0, :]
            B_sb = c_sb[:, 1, :]
            I_sb = c_sb[:, 2, :]

            out_sb = out_pool.tile([128, ROWS_PER_CORE * NJ], dt.float32)

            for r in range(ROWS_PER_CORE):
                ot_sb = io_pool.tile([128, 2, F], dt.float32, tag="ot")
                nc.sync.dma_start(ot_sb[:], ot4[r])

                d_sb = dpool.tile([128, F], dt.float32, tag="d")
                nc.vector.tensor_sub(d_sb[:], ot_sb[:, 0, :], ot_sb[:, 1, :])

                xb = xbpool.tile([128, XBW], dt.float32, tag="xb")
                # 16 PE transposes, batched 4 per PSUM bank, one copy per bank
                for g in range(NT // 4):
                    ptr = ptr_pool.tile([128, 512], dt.float32, tag="tr")
                    for q in range(4):
                        tt = 4 * g + q
                        nc.tensor.transpose(
                            ptr[:, 128 * q : 128 * (q + 1)],
                            d_sb[:, 128 * tt : 128 * (tt + 1)],
                            I_sb[:],
                        )
                    dst = xb[:, 128 + 512 * g : 128 + 512 * (g + 1)]
                    nc.vector.tensor_copy(dst, ptr[:])
                # prev-block pad: col p holds block (p-1, tt=15) = data col 2047+p
                nc.vector.memset(xb[:, 0:1], 0.0)
                nc.vector.tensor_copy(xb[:, 1:128], xb[:, 2048 : 2048 + 127])

                for j in range(NJ):
                    py = pconv_pool.tile([128, 512], dt.float32, tag="y")
                    nc.tensor.matmul(
                        py[:],
                        A_sb[:],
                        xb[:, 128 + 512 * j : 128 + 512 * (j + 1)],
                        start=True,
                        stop=False,
                    )
                    nc.tensor.matmul(
                        py[:],
                        B_sb[:],
                        xb[:, 512 * j : 512 * (j + 1)],
                        start=False,
                        stop=True,
                    )
                    col = NJ * r + j
                    acc = out_sb[:, col : col + 1]
                    scr = scr_pool.tile([128, 512], dt.float32, tag="scr")
                    nc.scalar.activation(
                        scr[:],
                        py[:],
                        mybir.ActivationFunctionType.Square,
                        accum_out=acc,
                    )

            nc.sync.dma_start(out_h[:], out_sb[:])

    _drop_vacuous_self_waits(nc)
    return nc


def kernel(output, target, b, a):
    global last_exec_time_ns
    from concourse.bass_utils import run_bass_kernel_spmd

    output = np.asarray(output, np.float32)
    target = np.asarray(target, np.float32)

    if "nc" not in _CACHE:
        _CACHE["nc"] = _build_bass()
    nc = _CACHE["nc"]

    h = _impulse_response(np.asarray(b, np.float64), np.asarray(a, np.float64), K)
    A_m, B_m = _toeplitz_lhsts(h)
    consts = np.stack([A_m, B_m, np.eye(K, dtype=np.float32)])

    ot = np.stack([output, target], axis=1)  # [B, 2, T]
    in_maps = []
    for c in range(NCORES):
        rows = slice(c * ROWS_PER_CORE, (c + 1) * ROWS_PER_CORE)
        in_maps.append(
            {
                "ot": np.ascontiguousarray(ot[rows]),
                "consts": consts,
            }
        )

    res = run_bass_kernel_spmd(
        nc,
        in_maps,
        core_ids=list(range(NCORES)),
        trace=bool(int(os.environ.get("LP_TRACE", "0"))),
    )
    last_exec_time_ns = res.exec_time_ns

    total = np.float64(0.0)
    for r in res.results:
        total += r["partials"].astype(np.float64).sum()
    return np.float32(total / (B * T))



# revision 17
# speedup vs baseline: 1.3405x; 1.3405x over previous
"""LowPassMSELoss Trainium2 kernel (v2: chunked DMA pipeline + fp16 PE path).

Math: loss = mean((lfilter(b,a,o) - lfilter(b,a,t))^2)
    = mean(lfilter(b,a,o-t)^2)               [filter is linear]
    = mean(conv(o-t, h)^2)                   [h = impulse response, truncated
                                              to K=128 taps; max pole radius
                                              0.869 -> tail < 2e-8]

Per core (2 rows of T=262144), per row:
  - 4 input DMA chunks [128, 2, 512] fp32 (natural layout: partition p =
    contiguous 2048-sample span, f-sliced into 512-col chunks)
  - d = (o - t) cast to fp16 on DVE, per chunk
  - 4 transposes per chunk as plain fp16 matmuls (lhsT = d block, rhs = I):
    xb data col 128 + 128*tt + p holds 128-sample block (16p + tt)
  - pad cols [0,128): previous block for tt=0 (copied from tt=15 cols;
    col 0 zeroed = zero filter state at row start)
  - conv tile j (=chunk index): psum[jj,n] = sum_i A[i,jj] xb_cur[i,n]
    + sum_i B[i,jj] xb_prev[i,n], Toeplitz lhsT from h (scaled 16x, fp16);
    tile j=0 needs the pad -> runs after chunk 3
  - square+reduce per psum tile on ACT (in-place) -> per-partition partials
  - host: sum partials over 8 cores / (16*262144*256)
"""

import os
import numpy as np

B, T = 16, 262144
NCORES = 8
ROWS_PER_CORE = B // NCORES          # 2
F = 2048                             # free dim of natural layout (T / 128)
K = 128                              # FIR taps
NJ = F // 512                        # 4 conv tiles (= chunks) per row
XBW = 128 + F                        # xb width (128 pad cols + data)
HSCALE = 16.0                        # keep fp16 taps in normal range

last_exec_time_ns = None
_CACHE = {}


def _impulse_response(b, a, n):
    """First n samples of the IIR impulse response, float64, DF2T like scipy."""
    b = np.asarray(b, np.float64)
    a = np.asarray(a, np.float64)
    b = b / a[0]
    a = a / a[0]
    order = len(a) - 1
    z = np.zeros(order, np.float64)
    h = np.empty(n, np.float64)
    for i in range(n):
        x = 1.0 if i == 0 else 0.0
        y = b[0] * x + z[0]
        znew = np.empty(order, np.float64)
        znew[: order - 1] = z[1:] + b[1:order] * x - a[1:order] * y
        znew[order - 1] = b[order] * x - a[order] * y
        z = znew
        h[i] = y
    return h


def _toeplitz_lhsts(h):
    """lhsT_A[i,j] = h[j-i] (j>=i), lhsT_B[i,j] = h[128+j-i] (i>j).

    y[128n+j] = sum_{i<=j} h[j-i]*cur[i] + sum_{i>j} h[128+j-i]*prev[i]
    matmul(out, lhsT, rhs): out[j, n] = sum_i lhsT[i, j] * rhs[i, n]
    """
    i = np.arange(K)[:, None]
    j = np.arange(K)[None, :]
    dj = j - i
    A = np.where(dj >= 0, h[np.clip(dj, 0, K - 1)], 0.0)
    Bm = np.where(dj < 0, h[np.clip(K + dj, 0, K - 1)], 0.0)
    return A, Bm


def _drop_vacuous_self_waits(nc):
    """trn2 codegen allows one sync-wait per instruction; Tile sometimes
    attaches a same-engine self-wait alongside a foreign one.  Engine queues
    issue in order and every same-engine op increments the engine sem, so a
    self-wait whose threshold is already guaranteed by queue position is
    droppable."""
    import copy

    prior_incs = {}
    for f in nc.m.functions:
        for bb in f.blocks:
            new_list = []
            for ins in bb.instructions:
                si = ins.sync_info
                if (
                    si is not None
                    and si.on_wait
                    and len(si.on_wait) > 1
                    and "Drain" in type(ins).__name__
                ):
                    waits = list(si.on_wait)
                    for k, w in enumerate(waits[:-1]):
                        pre = copy.deepcopy(ins)
                        pre.name = f"{ins.name}-w{k}"
                        pre.sync_info = copy.deepcopy(si)
                        pre.sync_info.on_wait = [w]
                        pre.sync_info.on_update = []
                        new_list.append(pre)
                    si.on_wait = [waits[-1]]
                new_list.append(ins)
            bb.instructions = new_list
    for f in nc.m.functions:
        for bb in f.blocks:
            for ins in bb.instructions:
                si = ins.sync_info
                if si is None:
                    continue
                waits = list(si.on_wait or [])
                if len(waits) > 1:
                    kept = []
                    for w in waits:
                        name = getattr(w, "ant_name", "") or ""
                        eng = getattr(getattr(ins, "engine", None), "value", "zz")
                        if (
                            name.startswith(eng)
                            and prior_incs.get(name, 0) >= (w.wait_value or 0)
                        ):
                            continue
                        kept.append(w)
                    si.on_wait = kept
                for u in si.on_update or []:
                    name = getattr(u, "ant_name", "") or ""
                    if name:
                        prior_incs[name] = prior_incs.get(name, 0) + (
                            u.update_value or 1
                        )


def _build_bass():
    import concourse.bass as bass
    import concourse.tile as tile
    from concourse import mybir

    dt = mybir.dt
    nc = bass.Bass(trn_type="TRN2")

    ot_h = nc.dram_tensor(
        "ot", [ROWS_PER_CORE, 2, T], dt.float32, kind="ExternalInput"
    )
    C_h = nc.dram_tensor("consts", [3, K, K], dt.float16, kind="ExternalInput")
    out_h = nc.dram_tensor(
        "partials", [128, ROWS_PER_CORE * NJ], dt.float32, kind="ExternalOutput"
    )

    # ot4[r, p, s, f] = ot[r, s, 2048p + f]
    ot4 = ot_h[:].rearrange("b s (p f) -> b p s f", p=128)

    with tile.TileContext(nc) as tc:
        with (
            tc.tile_pool(name="consts", bufs=1) as consts,
            tc.tile_pool(name="io", bufs=2 * NJ * ROWS_PER_CORE) as io_pool,
            tc.tile_pool(name="dpool", bufs=3) as dpool,
            tc.tile_pool(name="xb", bufs=ROWS_PER_CORE) as xbpool,
            tc.tile_pool(name="ptr", bufs=2, space="PSUM") as ptr_pool,
            tc.tile_pool(name="ya", bufs=4, space="PSUM") as ya_pool,
            tc.tile_pool(name="yd", bufs=2, space="PSUM") as yd_pool,
            tc.tile_pool(name="scr", bufs=4) as scr_pool,
            tc.tile_pool(name="outp", bufs=1) as out_pool,
        ):
            # input chunks first on the queue: data starts flowing ASAP
            io_tiles = {}
            pad_tiles = {}
            first = True
            for r in range(ROWS_PER_CORE):
                for c in range(NJ):
                    t_io = io_pool.tile([128, 2, 512], dt.float32, tag="ot")
                    nc.sync.dma_start(t_io[:], ot4[r][:, :, 512 * c : 512 * (c + 1)])
                    io_tiles[(r, c)] = t_io
                    if first:
                        # pad source: last 128 samples of every 2048-chunk,
                        # re-fetched small so tile j=0 needn't wait for chunk 3
                        for rr in range(ROWS_PER_CORE):
                            t_pad = io_pool.tile(
                                [128, 2, 128], dt.float32, tag="pad", name="pad"
                            )
                            nc.sync.dma_start(
                                t_pad[:], ot4[rr][:, :, 1920:2048]
                            )
                            pad_tiles[rr] = t_pad
                        c_raw = consts.tile([K, 3, K], dt.float16, tag="Craw")
                        nc.sync.dma_start(
                            c_raw[:], C_h[:].rearrange("c p f -> p c f")
                        )
                        first = False
            # funnel the const-DMA dep through DVE so PE ops wait on one engine
            c_sb = consts.tile([K, 3, K], dt.float16, tag="C")
            nc.vector.tensor_copy(c_sb[:], c_raw[:])
            A_sb = c_sb[:, 0, :]
            B_sb = c_sb[:, 1, :]
            I_sb = c_sb[:, 2, :]

            out_sb = out_pool.tile([128, ROWS_PER_CORE * NJ], dt.float32)

            # Half the y-tiles (ya) get dedicated psum banks (bufs=4, never
            # recycled -> no WAR wait at all) and are squared by ACT in place.
            # The other half (yd) recycle 2 banks; their only psum reader is a
            # DVE copy to SBUF, so the recycling matmul's WAR wait lands on
            # the DVE sem and merges with its DVE data wait (HW allows one
            # sync-wait per matmul).  ACT squares those from SBUF.
            tile_ct = [0]

            def conv_tile(py, col, mybir=mybir, nc=nc, out_sb=out_sb):
                acc = out_sb[:, col : col + 1]
                if tile_ct[0] % 2 == 0:
                    nc.scalar.activation(
                        py[:],
                        py[:],
                        mybir.ActivationFunctionType.Square,
                        scale=1.0 / HSCALE,
                        accum_out=acc,
                    )
                else:
                    scr = scr_pool.tile(
                        [128, 512], dt.float16, tag="scr", name="scr"
                    )
                    nc.vector.tensor_copy(scr[:], py[:])
                    nc.scalar.activation(
                        scr[:],
                        scr[:],
                        mybir.ActivationFunctionType.Square,
                        scale=1.0 / HSCALE,
                        accum_out=acc,
                    )
                tile_ct[0] += 1

            def y_tile():
                pool = ya_pool if tile_ct[0] % 2 == 0 else yd_pool
                tag = "ya" if tile_ct[0] % 2 == 0 else "yd"
                return pool.tile([128, 512], dt.float32, tag=tag, name=tag)

            for r in range(ROWS_PER_CORE):
                xb = xbpool.tile([128, XBW], dt.float16, tag="xb")

                # pad: col p holds block 16p-1 (= tile tt=15 of chunk p-1);
                # col 0 = zeros (zero filter state at row start).  Built from
                # the small re-fetched pad DMA so tile j=0 is not deferred.
                t_pad = pad_tiles[r]
                d16p = dpool.tile([128, 128], dt.float16, tag="dp", name="dp")
                nc.vector.tensor_sub(d16p[:], t_pad[:, 0, :], t_pad[:, 1, :])
                ptrp = ptr_pool.tile([128, 128], dt.float32, tag="tr", name="trp")
                nc.tensor.matmul(
                    ptrp[:], d16p[:], I_sb[:], start=True, stop=True
                )
                nc.vector.memset(xb[:, 0:1], 0.0)
                nc.vector.tensor_copy(xb[:, 1:128], ptrp[:, 0:127])

                for c in range(NJ):
                    t_io = io_tiles[(r, c)]
                    d16 = dpool.tile([128, 512], dt.float16, tag="d")
                    nc.vector.tensor_sub(d16[:], t_io[:, 0, :], t_io[:, 1, :])

                    ptr = ptr_pool.tile([128, 512], dt.float32, tag="tr")
                    for q in range(4):
                        nc.tensor.matmul(
                            ptr[:, 128 * q : 128 * (q + 1)],
                            d16[:, 128 * q : 128 * (q + 1)],
                            I_sb[:],
                            start=True,
                            stop=True,
                        )
                    dst = xb[:, 128 + 512 * c : 128 + 512 * (c + 1)]
                    nc.vector.tensor_copy(dst, ptr[:])

                    py = y_tile()
                    nc.tensor.matmul(
                        py[:],
                        A_sb[:],
                        xb[:, 128 + 512 * c : 128 + 512 * (c + 1)],
                        start=True,
                        stop=False,
                    )
                    nc.tensor.matmul(
                        py[:],
                        B_sb[:],
                        xb[:, 512 * c : 512 * (c + 1)],
                        start=False,
                        stop=True,
                    )
                    conv_tile(py, NJ * r + c)

            # issue from ACT's HWDGE queue: the dep on ACT's accum writes is
            # implicit in program order, keeping this under the 1-wait limit
            nc.scalar.dma_start(out_h[:], out_sb[:])

    _drop_vacuous_self_waits(nc)
    return nc


def kernel(output, target, b, a):
    global last_exec_time_ns
    from concourse.bass_utils import run_bass_kernel_spmd

    output = np.asarray(output, np.float32)
    target = np.asarray(target, np.float32)

    if "nc" not in _CACHE:
        _CACHE["nc"] = _build_bass()
    nc = _CACHE["nc"]

    h = _impulse_response(np.asarray(b, np.float64), np.asarray(a, np.float64), K)
    A_m, B_m = _toeplitz_lhsts(h * HSCALE)
    consts = np.stack([A_m, B_m, np.eye(K)]).astype(np.float16)

    ot = np.stack([output, target], axis=1)  # [B, 2, T]
    in_maps = []
    for c in range(NCORES):
        rows = slice(c * ROWS_PER_CORE, (c + 1) * ROWS_PER_CORE)
        in_maps.append(
            {
                "ot": np.ascontiguousarray(ot[rows]),
                "consts": consts,
            }
        )

    res = run_bass_kernel_spmd(
        nc,
        in_maps,
        core_ids=list(range(NCORES)),
        trace=bool(int(os.environ.get("LP_TRACE", "0"))),
    )
    last_exec_time_ns = res.exec_time_ns

    total = np.float64(0.0)
    for r in res.results:
        total += r["partials"].astype(np.float64).sum()
    # squares are descaled by 1/HSCALE inside the ACT (scale applies pre-func)
    return np.float32(total / (B * T))
